# revision 8
# baseline (speedup 1.0000x reference)
"""GEMM + reduce-scatter (nn_GemmRSIntraNode) as a Bass kernel on 8 trn2 cores.

Full semantics: out = einsum('rmk,rnk->mn', input, weight).reshape(8, 1024, 4096)
with input [8, 8192, 1024] f32 and weight [8, 4096, 1024] f32.

Sharding choice: instead of mimicking the per-rank partial-GEMM +
reduce-scatter, each core c directly computes output rows
[c*1024:(c+1)*1024] of the reduced result:

    out_c = sum_{r,k} input[r, c*1024:(c+1)*1024, k] * weight[r, n, k]

i.e. a [1024, 8192] x [8192, 4096] GEMM per core where the contraction
axis is (r, k) flattened. The rank-sum IS the K-axis contraction, so no
cross-core communication is needed at all; the "reduce-scatter" is
absorbed into the GEMM. Inputs are pre-transposed host-side into
[K, M] / [K, N] layouts so the device kernel does only contiguous DMA
and matmuls.

Compute dtype is bf16 (host-side rounding; norm rel err ~2.3e-3 with f32
PSUM accumulation, vs the 2e-2 gate). This leaves the whole [8192, 1024]
A slice SBUF-resident (16 MB of 26) and gives the B stream >2x DMA
slack, so the kernel is purely PE-bound.

The device kernel is raw bass (no Tile framework) with hand-rolled
coarse semaphores: measured per-matmul overhead on this part is ~16 ns
per LDWEIGHTS + ~6 ns fixed at a ~2.1 GHz sustained PE clock, so the
kernel shares each stationary operand across 2 matmuls (NSUB=2) and
ticks its progress semaphore once per k-chunk instead of per
instruction. Measured body time 1.096 ms/core vs the 0.999 ms pure
streaming floor (4096 matmuls x 512 cols at 2.1 GHz).

Loop nest: psum groups of 4 banks [2 m-subtiles x 2 n-subtiles],
ping-ponging between bank sets 0-3/4-7; MH=4 m-passes x NBLK=4 n-blocks
per pass; B streamed once per m-pass (4x total, 256 MB, still <50% DMA
duty). B tiles ride an 8-slot SBUF ring; each ring slot has its own
completion semaphore (completions of distinct in-flight DMAs are
unordered, so shared counting semaphores are ambiguous - Tile rotates 8
DMAHW sems for the same reason).

Engine programs:
  scalar : A tile kc -> [a_sems[kc%8] self-throttle], load, +16;
           then out DMA j: wait dve_sem >= j+1, c->HBM, +16 o_sems[j%8]
  sync   : B tile t -> wait pe_sem >= t-(RB-1), load, +16 b_sems[t%RB]
  tensor : kc-iter t: [dve_sem >= 4*(G-1) at group starts]
           [pass 0: a_sems[kc%8] >= 16*(kc//8+1)]
           wait b_sems[t%RB] >= 16*(t//RB+1), 2x(LDW + 2 matmuls),
           last matmul +1 pe_sem
  vector : group G bank bi (j = 4G+bi): wait pe_sem >= 64(G+1),
           [o_sems[j%8] >= 16*(j//8) for c-ring reuse],
           copy psum->c, +1 dve_sem
"""

import os
from contextlib import ExitStack

import numpy as np

WS = 8
M = 8192
N = 4096
LK = 1024
K = WS * LK          # 8192 contraction (rank*local_k)
M_LOC = M // WS      # 1024 output rows per core
N_CORES = 8
KC = K // 128        # 64 k-chunks

DTYPE = os.environ.get("BASS_KERNEL_DTYPE", "bfloat16")

_NC_CACHE = {}


def _build_nc_bf16_raw(reps=1):
    """Raw-bass bf16 kernel; see module docstring for the design."""
    from concourse import bacc, mybir

    f32 = mybir.dt.float32
    bf16 = mybir.dt.bfloat16

    NSUB = int(os.environ.get("BASS_RAW_NSUB", "2"))  # matmuls per ldweights
    MSUB = 4 // NSUB             # psum group = MSUB x NSUB = 4 banks
    BANKS = MSUB * NSUB
    MH = M_LOC // (MSUB * 128)   # m passes over B
    NBW = NSUB * 512             # B tile width
    NBLK = N // NBW              # n-blocks per pass
    GROUPS = MH * NBLK           # 16 per rep
    RB = int(os.environ.get("BASS_RAW_RB", "24"))  # B-tile ring slots
    DMA_INC = 16                 # sem bump per completed dma_start

    nc = bacc.Bacc("TRN2", target_bir_lowering=False, debug=False,
                   num_devices=N_CORES)
    a_d = nc.dram_tensor("a", [K, M_LOC], bf16, kind="ExternalInput")
    b_d = nc.dram_tensor("b", [K, N], bf16, kind="ExternalInput")
    o_d = nc.dram_tensor("o", [M_LOC, N], f32, kind="ExternalOutput")

    n_groups = GROUPS * reps
    n_iters = n_groups * KC      # kc-iterations (4 matmuls each)

    with ExitStack() as ctx:
        a_sb = ctx.enter_context(nc.sbuf_tensor("a_sb", [128, KC * M_LOC],
                                                bf16))
        b_sb = ctx.enter_context(nc.sbuf_tensor("b_sb", [128, RB * NBW],
                                                bf16))
        c_sb = ctx.enter_context(nc.sbuf_tensor("c_sb", [128, 8 * 512], f32))
        psum = [ctx.enter_context(nc.psum_tensor(f"ps{i}", [128, 512], f32))
                for i in range(8)]
        a_sems = [ctx.enter_context(nc.semaphore(name=f"a_sem{i}"))
                  for i in range(8)]
        b_sems = [ctx.enter_context(nc.semaphore(name=f"b_sem{i}"))
                  for i in range(RB)]
        o_sems = [ctx.enter_context(nc.semaphore(name=f"o_sem{i}"))
                  for i in range(8)]
        pe_sem = ctx.enter_context(nc.semaphore(name="pe_sem"))
        dve_sem = ctx.enter_context(nc.semaphore(name="dve_sem"))
        block = ctx.enter_context(nc.Block())

        def giter(t):
            g_abs = t // KC
            kc = t % KC
            g = g_abs % GROUPS
            return g_abs, g // NBLK, g % NBLK, kc

        @block.scalar
        def _(scalar):
            for kc in range(KC):
                if kc >= 8:
                    # keep one in-flight DMA per semaphore
                    scalar.wait_ge(a_sems[kc % 8], DMA_INC * (kc // 8))
                scalar.dma_start(
                    a_sb[:, kc * M_LOC:(kc + 1) * M_LOC],
                    a_d.ap()[kc * 128:(kc + 1) * 128, :],
                ).then_inc(a_sems[kc % 8], 16)
            # after A is resident, the scalar queue drains the output:
            # one DMA per finished psum-bank copy
            for j in range(n_groups * BANKS):
                g_abs, bi = j // BANKS, j % BANKS
                ms, ns = bi // NSUB, bi % NSUB
                g = g_abs % GROUPS
                mh, nb = g // NBLK, g % NBLK
                cs = j % 8
                row0 = mh * (MSUB * 128) + ms * 128
                col0 = nb * NBW + ns * 512
                scalar.wait_ge(dve_sem, j + 1)
                scalar.dma_start(
                    o_d.ap()[row0:row0 + 128, col0:col0 + 512],
                    c_sb[:, cs * 512:(cs + 1) * 512],
                ).then_inc(o_sems[j % 8], 16)

        @block.sync
        def _(sync):
            for t in range(n_iters):
                _, _, nb, kc = giter(t)
                if t >= RB:
                    sync.wait_ge(pe_sem, t - (RB - 1))
                slot = t % RB
                sync.dma_start(
                    b_sb[:, slot * NBW:(slot + 1) * NBW],
                    b_d.ap()[kc * 128:(kc + 1) * 128,
                             nb * NBW:(nb + 1) * NBW],
                ).then_inc(b_sems[slot], 16)

        @block.tensor
        def _(tensor):
            for t in range(n_iters):
                g_abs, mh, nb, kc = giter(t)
                bank0 = (g_abs % 2) * BANKS
                if kc == 0 and g_abs >= 2:
                    tensor.wait_ge(dve_sem, BANKS * (g_abs - 1))
                if g_abs == 0:
                    tensor.wait_ge(a_sems[kc % 8], DMA_INC * (kc // 8 + 1))
                slot = t % RB
                tensor.wait_ge(b_sems[slot], DMA_INC * (t // RB + 1))
                for ms in range(MSUB):
                    col = kc * M_LOC + mh * (MSUB * 128) + ms * 128
                    lhsT = a_sb[:, col:col + 128]
                    for ns in range(NSUB):
                        rhs = b_sb[:, slot * NBW + ns * 512:
                                   slot * NBW + ns * 512 + 512]
                        mm = tensor.matmul(psum[bank0 + ms * NSUB + ns][:, :],
                                           lhsT, rhs,
                                           start=(kc == 0),
                                           stop=(kc == KC - 1))
                        if ms == MSUB - 1 and ns == NSUB - 1:
                            mm.then_inc(pe_sem, 1)

        @block.vector
        def _(vector):
            for j in range(n_groups * BANKS):
                g_abs, bi = j // BANKS, j % BANKS
                bank0 = (g_abs % 2) * BANKS
                if bi == 0:
                    vector.wait_ge(pe_sem, KC * (g_abs + 1))
                if j >= 8:
                    # c-ring slot reuse: out DMA j-8 must have completed
                    vector.wait_ge(o_sems[j % 8], DMA_INC * (j // 8))
                cs = j % 8
                vector.tensor_copy(
                    c_sb[:, cs * 512:(cs + 1) * 512],
                    psum[bank0 + bi][:, :],
                ).then_inc(dve_sem, 1)

    nc.compile()
    return nc


def _build_nc_f32(dt_name):
    """Tile-framework f32/f32r fallback (the original kernel)."""
    import concourse.tile as tile
    from concourse import bacc, mybir

    f32 = mybir.dt.float32
    if dt_name == "float32r":
        # fp32r: PE rounds operands to 11-bit mantissa internally; feeding
        # raw f32 bits declared f32r is bit-identical to a casting DMA
        # (verified on hw), so declare DRAM f32r and use non-cast DMAs.
        io_dt = mybir.dt.float32r
        sb_dt = mybir.dt.float32r
    elif dt_name == "float32":
        io_dt = f32
        sb_dt = f32
    else:
        raise ValueError(dt_name)

    M_RES = 512        # half of A resident per m-block
    NSUB = int(os.environ.get("BASS_NSUB", "2"))
    MSUB = M_RES // 128
    assert MSUB * NSUB <= 8
    NBW = NSUB * 512
    NBLK = N // NBW
    MBLK = M_LOC // M_RES
    A_BUFS = KC + 12
    B_BUFS = 8
    REPS = int(os.environ.get("BASS_REPS", "1"))

    nc = bacc.Bacc("TRN2", target_bir_lowering=False, debug=False,
                   num_devices=N_CORES)
    a_d = nc.dram_tensor("a", [K, M_LOC], io_dt, kind="ExternalInput")
    b_d = nc.dram_tensor("b", [K, N], io_dt, kind="ExternalInput")
    o_d = nc.dram_tensor("o", [M_LOC, N], f32, kind="ExternalOutput")

    with tile.TileContext(nc) as tc, ExitStack() as ctx:
        apool = ctx.enter_context(tc.tile_pool(name="apool", bufs=A_BUFS))
        bpool = ctx.enter_context(tc.tile_pool(name="bpool", bufs=B_BUFS))
        cpool = ctx.enter_context(tc.tile_pool(name="cpool", bufs=8))
        pp = ctx.enter_context(tc.tile_pool(name="pp", bufs=8, space="PSUM"))

        def load_a(mb, kc):
            a_t = apool.tile([128, M_RES], sb_dt, name=f"a_{mb}_{kc}",
                             tag="a")
            nc.sync.dma_start(
                a_t[:],
                a_d.ap()[kc * 128:(kc + 1) * 128,
                         mb * M_RES:(mb + 1) * M_RES])
            return a_t

        a_tiles = [load_a(0, kc) for kc in range(KC)]
        for mbi in range(MBLK * REPS):
            mb = mbi % MBLK
            next_a = [] if mbi + 1 < MBLK * REPS else None
            mb_next = (mbi + 1) % MBLK
            for nb in range(NBLK):
                last_nb = nb == NBLK - 1
                psums = [[pp.tile([128, 512], f32,
                                  name=f"p_{mb}_{nb}_{ms}_{ns}", tag="p")
                          for ns in range(NSUB)] for ms in range(MSUB)]
                for kc in range(KC):
                    b_t = bpool.tile([128, NBW], sb_dt,
                                     name=f"b_{mb}_{nb}_{kc}", tag="b")
                    nc.sync.dma_start(
                        b_t[:],
                        b_d.ap()[kc * 128:(kc + 1) * 128,
                                 nb * NBW:(nb + 1) * NBW])
                    for ms in range(MSUB):
                        lhsT = a_tiles[kc][:, ms * 128:(ms + 1) * 128]
                        for ns in range(NSUB):
                            rhs = b_t[:, ns * 512:(ns + 1) * 512]
                            nc.tensor.matmul(psums[ms][ns][:], lhsT, rhs,
                                             start=(kc == 0),
                                             stop=(kc == KC - 1))
                    if last_nb and next_a is not None:
                        next_a.append(load_a(mb_next, kc))
                for ms in range(MSUB):
                    for ns in range(NSUB):
                        row0 = mb * M_RES + ms * 128
                        col0 = nb * NBW + ns * 512
                        c_t = cpool.tile([128, 512], f32,
                                         name=f"c_{mb}_{nb}_{ms}_{ns}",
                                         tag="c")
                        nc.vector.tensor_copy(c_t[:], psums[ms][ns][:])
                        nc.sync.dma_start(
                            o_d.ap()[row0:row0 + 128, col0:col0 + 512],
                            c_t[:])
            if next_a is not None:
                a_tiles = next_a

    nc.compile()
    return nc


def _build_nc(dt_name):
    if dt_name == "bfloat16":
        return _build_nc_bf16_raw(int(os.environ.get("BASS_REPS", "1")))
    return _build_nc_f32(dt_name)


def get_nc(dt_name=None):
    dt_name = dt_name or DTYPE
    if dt_name not in _NC_CACHE:
        _NC_CACHE[dt_name] = _build_nc(dt_name)
    return _NC_CACHE[dt_name]


def make_in_maps(input, weight, dt_name=None):
    """Host-side shard + layout prep. Returns in_maps for cores 0..7."""
    dt_name = dt_name or DTYPE
    input = np.asarray(input, dtype=np.float32)
    weight = np.asarray(weight, dtype=np.float32)
    assert input.shape == (WS, M, LK), input.shape
    assert weight.shape == (WS, N, LK), weight.shape

    if dt_name == "bfloat16":
        import ml_dtypes
        np_dt = ml_dtypes.bfloat16
    else:
        np_dt = np.float32

    # B[r*LK + k, n] = weight[r, n, k]  -> [K, N]
    b_full = np.ascontiguousarray(
        weight.transpose(0, 2, 1).reshape(K, N).astype(np_dt))
    in_maps = []
    for c in range(N_CORES):
        # A_c[r*LK + k, m] = input[r, c*M_LOC + m, k]  -> [K, M_LOC]
        a_c = np.ascontiguousarray(
            input[:, c * M_LOC:(c + 1) * M_LOC, :]
            .transpose(0, 2, 1).reshape(K, M_LOC).astype(np_dt))
        in_maps.append({"a": a_c, "b": b_full})
    return in_maps


def kernel(input, weight):
    from concourse import bass_utils

    nc = get_nc()
    in_maps = make_in_maps(input, weight)
    res = bass_utils.run_bass_kernel_spmd(
        nc, in_maps, core_ids=list(range(N_CORES)))
    out = np.stack([res.results[c]["o"] for c in range(N_CORES)], axis=0)
    return out.astype(np.float32)


# revision 9
# speedup vs baseline: 1.0187x; 1.0187x over previous
"""GEMM + reduce-scatter (nn_GemmRSIntraNode) as a Bass kernel on 8 trn2 cores.

Full semantics: out = einsum('rmk,rnk->mn', input, weight).reshape(8, 1024, 4096)
with input [8, 8192, 1024] f32 and weight [8, 4096, 1024] f32.

Sharding choice: instead of mimicking the per-rank partial-GEMM +
reduce-scatter, each core c directly computes output rows
[c*1024:(c+1)*1024] of the reduced result:

    out_c = sum_{r,k} input[r, c*1024:(c+1)*1024, k] * weight[r, n, k]

i.e. a [1024, 8192] x [8192, 4096] GEMM per core where the contraction
axis is (r, k) flattened. The rank-sum IS the K-axis contraction, so no
cross-core communication is needed at all; the "reduce-scatter" is
absorbed into the GEMM. Inputs are pre-transposed host-side into
[K, M] / [K, N] layouts so the device kernel does only contiguous DMA
and matmuls.

Compute dtype is bf16 (host-side rounding; norm rel err ~2.3e-3 with f32
PSUM accumulation, vs the 2e-2 gate). This leaves the whole [8192, 1024]
A slice SBUF-resident (16 MB of 26) and gives the B stream >2x DMA
slack, so the kernel is purely PE-bound.

The device kernel is raw bass (no Tile framework) with hand-rolled
coarse semaphores: measured per-matmul overhead on this part is ~16 ns
per LDWEIGHTS + ~6 ns fixed at a ~2.1 GHz sustained PE clock, so the
kernel shares each stationary operand across 2 matmuls (NSUB=2) and
ticks its progress semaphore once per k-chunk instead of per
instruction. Measured body time 1.096 ms/core vs the 0.999 ms pure
streaming floor (4096 matmuls x 512 cols at 2.1 GHz).

Loop nest: psum groups of 4 banks [2 m-subtiles x 2 n-subtiles],
ping-ponging between bank sets 0-3/4-7; MH=4 m-passes x NBLK=4 n-blocks
per pass; B streamed once per m-pass (4x total, 256 MB, still <50% DMA
duty). B tiles ride an 8-slot SBUF ring; each ring slot has its own
completion semaphore (completions of distinct in-flight DMAs are
unordered, so shared counting semaphores are ambiguous - Tile rotates 8
DMAHW sems for the same reason).

Engine programs:
  scalar : A tile kc -> [a_sems[kc%8] self-throttle], load, +16;
           then out DMA j: wait dve_sem >= j+1, c->HBM, +16 o_sems[j%8]
  sync   : B tile t -> wait pe_sem >= t-(RB-1), load, +16 b_sems[t%RB]
  tensor : kc-iter t: [dve_sem >= 4*(G-1) at group starts]
           [pass 0: a_sems[kc%8] >= 16*(kc//8+1)]
           wait b_sems[t%RB] >= 16*(t//RB+1), 2x(LDW + 2 matmuls),
           last matmul +1 pe_sem
  vector : group G bank bi (j = 4G+bi): wait pe_sem >= 64(G+1),
           [o_sems[j%8] >= 16*(j//8) for c-ring reuse],
           copy psum->c, +1 dve_sem
"""

import os
from contextlib import ExitStack

import numpy as np

WS = 8
M = 8192
N = 4096
LK = 1024
K = WS * LK          # 8192 contraction (rank*local_k)
M_LOC = M // WS      # 1024 output rows per core
N_CORES = 8
KC = K // 128        # 64 k-chunks

DTYPE = os.environ.get("BASS_KERNEL_DTYPE", "bfloat16")

_NC_CACHE = {}


def _build_nc_bf16_raw(reps=1):
    """Raw-bass bf16 kernel; see module docstring for the design."""
    from concourse import bacc, mybir

    f32 = mybir.dt.float32
    bf16 = mybir.dt.bfloat16

    NSUB = int(os.environ.get("BASS_RAW_NSUB", "2"))  # matmuls per ldweights
    MSUB = 4 // NSUB             # psum group = MSUB x NSUB = 4 banks
    BANKS = MSUB * NSUB
    MH = M_LOC // (MSUB * 128)   # m passes over B
    NBW = NSUB * 512             # B tile width
    NBLK = N // NBW              # n-blocks per pass
    GROUPS = MH * NBLK           # 16 per rep
    RB = int(os.environ.get("BASS_RAW_RB", "16"))  # B-tile ring slots
    DMA_INC = 16                 # sem bump per completed dma_start

    nc = bacc.Bacc("TRN2", target_bir_lowering=False, debug=False,
                   num_devices=N_CORES)
    a_d = nc.dram_tensor("a", [K, M_LOC], bf16, kind="ExternalInput")
    b_d = nc.dram_tensor("b", [K, N], bf16, kind="ExternalInput")
    o_d = nc.dram_tensor("o", [M_LOC, N], f32, kind="ExternalOutput")

    n_groups = GROUPS * reps
    n_iters = n_groups * KC      # kc-iterations (4 matmuls each)

    with ExitStack() as ctx:
        a_sb = ctx.enter_context(nc.sbuf_tensor("a_sb", [128, KC * M_LOC],
                                                bf16))
        b_sb = ctx.enter_context(nc.sbuf_tensor("b_sb", [128, RB * NBW],
                                                bf16))
        c_sb = ctx.enter_context(nc.sbuf_tensor("c_sb", [128, 8 * 512], f32))
        psum = [ctx.enter_context(nc.psum_tensor(f"ps{i}", [128, 512], f32))
                for i in range(8)]
        a_sems = [ctx.enter_context(nc.semaphore(name=f"a_sem{i}"))
                  for i in range(8)]
        b_sems = [ctx.enter_context(nc.semaphore(name=f"b_sem{i}"))
                  for i in range(RB)]
        o_sems = [ctx.enter_context(nc.semaphore(name=f"o_sem{i}"))
                  for i in range(8)]
        pe_sem = ctx.enter_context(nc.semaphore(name="pe_sem"))
        dve_sem = ctx.enter_context(nc.semaphore(name="dve_sem"))
        block = ctx.enter_context(nc.Block())

        def giter(t):
            g_abs = t // KC
            kc = t % KC
            g = g_abs % GROUPS
            return g_abs, g // NBLK, g % NBLK, kc

        @block.scalar
        def _(scalar):
            for kc in range(KC):
                if kc >= 8:
                    # keep one in-flight DMA per semaphore
                    scalar.wait_ge(a_sems[kc % 8], DMA_INC * (kc // 8))
                scalar.dma_start(
                    a_sb[:, kc * M_LOC:(kc + 1) * M_LOC],
                    a_d.ap()[kc * 128:(kc + 1) * 128, :],
                ).then_inc(a_sems[kc % 8], 16)
            # after A is resident, the scalar queue drains the output:
            # one DMA per finished psum-bank copy
            for j in range(n_groups * BANKS):
                g_abs, bi = j // BANKS, j % BANKS
                ms, ns = bi // NSUB, bi % NSUB
                g = g_abs % GROUPS
                mh, nb = g // NBLK, g % NBLK
                cs = j % 8
                row0 = mh * (MSUB * 128) + ms * 128
                col0 = nb * NBW + ns * 512
                scalar.wait_ge(dve_sem, j + 1)
                scalar.dma_start(
                    o_d.ap()[row0:row0 + 128, col0:col0 + 512],
                    c_sb[:, cs * 512:(cs + 1) * 512],
                ).then_inc(o_sems[j % 8], 16)

        @block.sync
        def _(sync):
            for t in range(n_iters):
                _, _, nb, kc = giter(t)
                if t >= RB:
                    sync.wait_ge(pe_sem, t - (RB - 1))
                slot = t % RB
                sync.dma_start(
                    b_sb[:, slot * NBW:(slot + 1) * NBW],
                    b_d.ap()[kc * 128:(kc + 1) * 128,
                             nb * NBW:(nb + 1) * NBW],
                ).then_inc(b_sems[slot], 16)

        @block.tensor
        def _(tensor):
            for t in range(n_iters):
                g_abs, mh, nb, kc = giter(t)
                bank0 = (g_abs % 2) * BANKS
                if kc == 0 and g_abs >= 2:
                    tensor.wait_ge(dve_sem, BANKS * (g_abs - 1))
                if g_abs == 0:
                    tensor.wait_ge(a_sems[kc % 8], DMA_INC * (kc // 8 + 1))
                slot = t % RB
                tensor.wait_ge(b_sems[slot], DMA_INC * (t // RB + 1))
                for ms in range(MSUB):
                    col = kc * M_LOC + mh * (MSUB * 128) + ms * 128
                    lhsT = a_sb[:, col:col + 128]
                    for ns in range(NSUB):
                        rhs = b_sb[:, slot * NBW + ns * 512:
                                   slot * NBW + ns * 512 + 512]
                        mm = tensor.matmul(psum[bank0 + ms * NSUB + ns][:, :],
                                           lhsT, rhs,
                                           start=(kc == 0),
                                           stop=(kc == KC - 1))
                        if ms == MSUB - 1 and ns == NSUB - 1:
                            mm.then_inc(pe_sem, 1)

        @block.vector
        def _(vector):
            for j in range(n_groups * BANKS):
                g_abs, bi = j // BANKS, j % BANKS
                bank0 = (g_abs % 2) * BANKS
                if bi == 0:
                    vector.wait_ge(pe_sem, KC * (g_abs + 1))
                if j >= 8:
                    # c-ring slot reuse: out DMA j-8 must have completed
                    vector.wait_ge(o_sems[j % 8], DMA_INC * (j // 8))
                cs = j % 8
                vector.tensor_copy(
                    c_sb[:, cs * 512:(cs + 1) * 512],
                    psum[bank0 + bi][:, :],
                ).then_inc(dve_sem, 1)

    nc.compile()
    return nc


def _build_nc_f32(dt_name):
    """Tile-framework f32/f32r fallback (the original kernel)."""
    import concourse.tile as tile
    from concourse import bacc, mybir

    f32 = mybir.dt.float32
    if dt_name == "float32r":
        # fp32r: PE rounds operands to 11-bit mantissa internally; feeding
        # raw f32 bits declared f32r is bit-identical to a casting DMA
        # (verified on hw), so declare DRAM f32r and use non-cast DMAs.
        io_dt = mybir.dt.float32r
        sb_dt = mybir.dt.float32r
    elif dt_name == "float32":
        io_dt = f32
        sb_dt = f32
    else:
        raise ValueError(dt_name)

    M_RES = 512        # half of A resident per m-block
    NSUB = int(os.environ.get("BASS_NSUB", "2"))
    MSUB = M_RES // 128
    assert MSUB * NSUB <= 8
    NBW = NSUB * 512
    NBLK = N // NBW
    MBLK = M_LOC // M_RES
    A_BUFS = KC + 12
    B_BUFS = 8
    REPS = int(os.environ.get("BASS_REPS", "1"))

    nc = bacc.Bacc("TRN2", target_bir_lowering=False, debug=False,
                   num_devices=N_CORES)
    a_d = nc.dram_tensor("a", [K, M_LOC], io_dt, kind="ExternalInput")
    b_d = nc.dram_tensor("b", [K, N], io_dt, kind="ExternalInput")
    o_d = nc.dram_tensor("o", [M_LOC, N], f32, kind="ExternalOutput")

    with tile.TileContext(nc) as tc, ExitStack() as ctx:
        apool = ctx.enter_context(tc.tile_pool(name="apool", bufs=A_BUFS))
        bpool = ctx.enter_context(tc.tile_pool(name="bpool", bufs=B_BUFS))
        cpool = ctx.enter_context(tc.tile_pool(name="cpool", bufs=8))
        pp = ctx.enter_context(tc.tile_pool(name="pp", bufs=8, space="PSUM"))

        def load_a(mb, kc):
            a_t = apool.tile([128, M_RES], sb_dt, name=f"a_{mb}_{kc}",
                             tag="a")
            nc.sync.dma_start(
                a_t[:],
                a_d.ap()[kc * 128:(kc + 1) * 128,
                         mb * M_RES:(mb + 1) * M_RES])
            return a_t

        a_tiles = [load_a(0, kc) for kc in range(KC)]
        for mbi in range(MBLK * REPS):
            mb = mbi % MBLK
            next_a = [] if mbi + 1 < MBLK * REPS else None
            mb_next = (mbi + 1) % MBLK
            for nb in range(NBLK):
                last_nb = nb == NBLK - 1
                psums = [[pp.tile([128, 512], f32,
                                  name=f"p_{mb}_{nb}_{ms}_{ns}", tag="p")
                          for ns in range(NSUB)] for ms in range(MSUB)]
                for kc in range(KC):
                    b_t = bpool.tile([128, NBW], sb_dt,
                                     name=f"b_{mb}_{nb}_{kc}", tag="b")
                    nc.sync.dma_start(
                        b_t[:],
                        b_d.ap()[kc * 128:(kc + 1) * 128,
                                 nb * NBW:(nb + 1) * NBW])
                    for ms in range(MSUB):
                        lhsT = a_tiles[kc][:, ms * 128:(ms + 1) * 128]
                        for ns in range(NSUB):
                            rhs = b_t[:, ns * 512:(ns + 1) * 512]
                            nc.tensor.matmul(psums[ms][ns][:], lhsT, rhs,
                                             start=(kc == 0),
                                             stop=(kc == KC - 1))
                    if last_nb and next_a is not None:
                        next_a.append(load_a(mb_next, kc))
                for ms in range(MSUB):
                    for ns in range(NSUB):
                        row0 = mb * M_RES + ms * 128
                        col0 = nb * NBW + ns * 512
                        c_t = cpool.tile([128, 512], f32,
                                         name=f"c_{mb}_{nb}_{ms}_{ns}",
                                         tag="c")
                        nc.vector.tensor_copy(c_t[:], psums[ms][ns][:])
                        nc.sync.dma_start(
                            o_d.ap()[row0:row0 + 128, col0:col0 + 512],
                            c_t[:])
            if next_a is not None:
                a_tiles = next_a

    nc.compile()
    return nc


def _build_nc(dt_name):
    if dt_name == "bfloat16":
        return _build_nc_bf16_raw(int(os.environ.get("BASS_REPS", "1")))
    return _build_nc_f32(dt_name)


def get_nc(dt_name=None):
    dt_name = dt_name or DTYPE
    if dt_name not in _NC_CACHE:
        _NC_CACHE[dt_name] = _build_nc(dt_name)
    return _NC_CACHE[dt_name]


def make_in_maps(input, weight, dt_name=None):
    """Host-side shard + layout prep. Returns in_maps for cores 0..7."""
    dt_name = dt_name or DTYPE
    input = np.asarray(input, dtype=np.float32)
    weight = np.asarray(weight, dtype=np.float32)
    assert input.shape == (WS, M, LK), input.shape
    assert weight.shape == (WS, N, LK), weight.shape

    if dt_name == "bfloat16":
        import ml_dtypes
        np_dt = ml_dtypes.bfloat16
    else:
        np_dt = np.float32

    # B[r*LK + k, n] = weight[r, n, k]  -> [K, N]
    b_full = np.ascontiguousarray(
        weight.transpose(0, 2, 1).reshape(K, N).astype(np_dt))
    in_maps = []
    for c in range(N_CORES):
        # A_c[r*LK + k, m] = input[r, c*M_LOC + m, k]  -> [K, M_LOC]
        a_c = np.ascontiguousarray(
            input[:, c * M_LOC:(c + 1) * M_LOC, :]
            .transpose(0, 2, 1).reshape(K, M_LOC).astype(np_dt))
        in_maps.append({"a": a_c, "b": b_full})
    return in_maps


def kernel(input, weight):
    from concourse import bass_utils

    nc = get_nc()
    in_maps = make_in_maps(input, weight)
    res = bass_utils.run_bass_kernel_spmd(
        nc, in_maps, core_ids=list(range(N_CORES)))
    out = np.stack([res.results[c]["o"] for c in range(N_CORES)], axis=0)
    return out.astype(np.float32)
